# revision 18
# baseline (speedup 1.0000x reference)
"""Trainium2 Bass kernel for LogWeightedDICELossMultiClass3D (v3).

Input: output (4,3,64,192,192) f32, masks (same), loss_threshold scalar.
Sharding: H=192 split into 8 slabs of 24 rows (one per core, 1-row halo
clamped on host). Device layout per core: 6 supertiles of 128 partitions
(= 2 volumes x 64 z), free dim = H-rows x 192 W flat.

Host ships o and m as bf16 (m is 0/1 so exact; o rounds at ~4e-3 which
perturbs only the threshold comparison for |o-thr|<2e-3 - well inside the
loss tolerance). Per supertile, reduced to per-(volume-half, z) partials:
  sum(m)            ACT copy+accum
  sum(o)            ACT copy+accum
  sum(o*m)          DVE TT mult -> q, summed by routed PE ones-matmul
  sum(ts==m)        DVE TT is_equal -> eq, summed by routed PE ones-matmul
  sum(sobel_edge)   DVE chain d=D_W(ts), u=S_H(d) (2 adds); PE single-pass
                    banded matmul grad=S_Z(u); ACT sigmoid(100g-50)+accum
ts = (o > thr) via DVE tensor_scalar (4x mode, exact 0/1 bf16).
All grad values are small ints -> bf16/psum arithmetic is exact.
eq/q sums accumulate across supertiles into persistent [12,512] PSUM via a
per-supertile routing stationary matrix (row 2s+r = volume 2s+r).
Host combines the tiny partials into the scalar loss.
"""

import numpy as np
import ml_dtypes

import concourse.bacc as bacc
import concourse.bass as bass
import concourse.tile as tile
from concourse import mybir
import concourse.bass_utils as _bu
from concourse.bass_utils import run_bass_kernel_spmd



F32 = mybir.dt.float32
BF16 = mybir.dt.bfloat16
ALU = mybir.AluOpType
ACTF = mybir.ActivationFunctionType

B, C, Z, H, W = 4, 3, 64, 192, 192
NV = B * C            # 12 volumes
NCORES = 8
HC = H // NCORES      # 24 H-rows per core
NS = NV // 2          # 6 supertiles (2 volumes each)
FH = HC + 2           # 26 rows incl halo
FW = FH * W           # 4992 free elements per partition (o / ts / d)
UW = (HC + 1) * W     # 4800 (u1: 25 rows)
CW = HC * W           # 4608 center free elements
C0 = W                # flat offset of center region (row 1)
SPAN = 1536           # grad chunk (3 per supertile), 3 psum banks each
VOX = Z * H * W

_CACHE = {}


def _band64():
    """[1,2,1] Z-smoothing with scipy 'reflect' (np symmetric) ends."""
    M = np.zeros((Z, Z), dtype=np.float64)
    for i in range(Z):
        M[i, i] = 2.0
        if i > 0:
            M[i, i - 1] += 1.0
        else:
            M[i, i] += 1.0
        if i < Z - 1:
            M[i, i + 1] += 1.0
        else:
            M[i, i] += 1.0
    return M


def _consts():
    Bz = _band64()
    blk = np.zeros((128, 128), dtype=np.float64)
    blk[:64, :64] = Bz
    blk[64:, 64:] = Bz
    bz = blk.astype(ml_dtypes.bfloat16)           # weights 1,2 - exact
    # routing matrix: block s is [128, 12] with col 2s+r = 1 on half r
    ones12 = np.zeros((128, 12 * NS), dtype=ml_dtypes.bfloat16)
    for s in range(NS):
        ones12[:64, 12 * s + 2 * s] = 1.0
        ones12[64:, 12 * s + 2 * s + 1] = 1.0
    return bz, ones12


def _build_program():
    nc = bacc.Bacc("TRN2", target_bir_lowering=False, debug=False,
                   num_devices=NCORES)
    o_d = nc.dram_tensor("o", [NV * Z, FW], BF16, kind="ExternalInput").ap()
    m_d = nc.dram_tensor("m", [NV * Z, CW], BF16, kind="ExternalInput").ap()
    thr_d = nc.dram_tensor("thr", [1, 1], F32, kind="ExternalInput").ap()
    bz_d = nc.dram_tensor("bz", [128, 128], BF16, kind="ExternalInput").ap()
    ones_d = nc.dram_tensor("ones12", [128, 12 * NS], BF16,
                            kind="ExternalInput").ap()
    part_d = nc.dram_tensor("partials", [128, 30], F32, kind="ExternalOutput").ap()
    eqs_d = nc.dram_tensor("eqs", [12, 512], F32, kind="ExternalOutput").ap()
    qs_d = nc.dram_tensor("qs", [12, 512], F32, kind="ExternalOutput").ap()

    from contextlib import ExitStack
    with tile.TileContext(nc) as tc, ExitStack() as ctx:
        consts = ctx.enter_context(tc.tile_pool(name="consts", bufs=1))
        io = ctx.enter_context(tc.tile_pool(name="io", bufs=2))
        mid = ctx.enter_context(tc.tile_pool(name="mid", bufs=1))
        xfer = ctx.enter_context(tc.tile_pool(name="xfer", bufs=2))
        slots = ctx.enter_context(tc.tile_pool(name="slots", bufs=1))
        gps = ctx.enter_context(tc.tile_pool(name="gps", bufs=2, space="PSUM"))
        ops = ctx.enter_context(tc.tile_pool(name="ops", bufs=1, space="PSUM"))

        # first supertile's o goes out before anything else (head latency);
        # gpsimd SWDGE triggers fire without queueing behind the sync stream
        o0_t = io.tile([128, FW], BF16, tag="o", name="o0")
        nc.gpsimd.dma_start(
            out=o0_t[0:64, :], in_=o_d[0:64, :])
        nc.gpsimd.dma_start(
            out=o0_t[64:128, :], in_=o_d[64:128, :])
        m0_t = io.tile([128, CW], BF16, tag="m", name="m0")
        nc.gpsimd.dma_start(out=m0_t, in_=m_d[0:128, :])

        thr_t = consts.tile([128, 1], F32)
        nc.gpsimd.dma_start(out=thr_t, in_=thr_d.to_broadcast([128, 1]))
        bz_t = consts.tile([128, 128], BF16)
        nc.default_dma_engine.dma_start(out=bz_t, in_=bz_d)
        ones_t = consts.tile([128, 12 * NS], BF16)
        nc.default_dma_engine.dma_start(out=ones_t, in_=ones_d)
        nbias_t = consts.tile([128, 1], F32)
        nc.vector.memset(nbias_t, -50.0)

        msum = slots.tile([128, NS], F32)
        osum = slots.tile([128, NS], F32)
        edgesum = slots.tile([128, 3 * NS], F32)
        eqp = ops.tile([12, 512], F32, name="eqp")
        qp = ops.tile([12, 512], F32, name="qp")

        for s in range(NS):
            fold = s % 2 == 1     # fold u2's S_H tap into a 2-pass matmul
            if s == 0:
                o_t, m_t = o0_t, m0_t
            else:
                o_t = io.tile([128, FW], BF16, tag="o", name=f"o{s}")
                nc.default_dma_engine.dma_start(
                    out=o_t[0:64, :], in_=o_d[128 * s:128 * s + 64, :])
                nc.default_dma_engine.dma_start(
                    out=o_t[64:128, :], in_=o_d[128 * s + 64:128 * (s + 1), :])
                m_t = io.tile([128, CW], BF16, tag="m", name=f"m{s}")
                nc.default_dma_engine.dma_start(
                    out=m_t, in_=m_d[128 * s:128 * (s + 1), :])

            ts_t = mid.tile([128, FW], BF16, tag="ts", name=f"ts{s}")
            d_t = mid.tile([128, FW], BF16, tag="d", name=f"d{s}")
            u1_t = mid.tile([128, UW], BF16, tag="u1", name=f"u1{s}")
            if not fold:
                u2_t = xfer.tile([128, CW], BF16, tag="u2", name=f"u2{s}")
            eq_t = xfer.tile([128, CW], BF16, tag="eq", name=f"eq{s}")
            q_t = xfer.tile([128, CW], BF16, tag="q", name=f"q{s}")
            last = s == NS - 1

            # supertile 0 runs per partition-half so DVE starts on the first
            # half of the very first o transfer
            halves = [(0, 64), (64, 128)] if s == 0 else [(0, 128)]
            for (p0, p1) in halves:
                # ts = (o > thr), bf16 0/1 (exact), incl halo rows  [DVE 4x]
                nc.vector.tensor_scalar(
                    out=ts_t[p0:p1], in0=o_t[p0:p1], scalar1=thr_t[p0:p1],
                    scalar2=None, op0=ALU.is_gt)

                if last:
                    # feed the trailing eq/q matmuls as early as possible
                    nc.vector.tensor_tensor(
                        out=eq_t[p0:p1], in0=ts_t[p0:p1, C0:C0 + CW],
                        in1=m_t[p0:p1], op=ALU.is_equal)
                    nc.vector.tensor_tensor(
                        out=q_t[p0:p1], in0=o_t[p0:p1, C0:C0 + CW],
                        in1=m_t[p0:p1], op=ALU.mult)

                # d = W-derivative of ts (symmetric boundary), natural layout
                ts3 = ts_t[p0:p1].rearrange("p (a b) -> p a b", b=W)
                d3 = d_t[p0:p1].rearrange("p (a b) -> p a b", b=W)
                nc.vector.tensor_tensor(
                    out=d3[:, :, 1:191], in0=ts3[:, :, 2:192],
                    in1=ts3[:, :, 0:190], op=ALU.subtract)
                nc.vector.tensor_tensor(
                    out=d3[:, :, 0:192:191], in0=ts3[:, :, 1:192:190],
                    in1=ts3[:, :, 0:191:190], op=ALU.subtract)

                # u1 = d[h]+d[h+1]; u2 = u1[h]+u1[h+1] (= S_H(d)) either on
                # DVE or folded into the S_Z matmul as two accumulating passes
                nc.vector.tensor_tensor(
                    out=u1_t[p0:p1], in0=d_t[p0:p1, 0:UW],
                    in1=d_t[p0:p1, W:UW + W], op=ALU.add)
                if not fold:
                    nc.vector.tensor_tensor(
                        out=u2_t[p0:p1], in0=u1_t[p0:p1, 0:CW],
                        in1=u1_t[p0:p1, W:CW + W], op=ALU.add)

                if not last:
                    nc.vector.tensor_tensor(
                        out=eq_t[p0:p1], in0=ts_t[p0:p1, C0:C0 + CW],
                        in1=m_t[p0:p1], op=ALU.is_equal)
                    nc.vector.tensor_tensor(
                        out=q_t[p0:p1], in0=o_t[p0:p1, C0:C0 + CW],
                        in1=m_t[p0:p1], op=ALU.mult)

            # sum(m), sum(o) on ACT (copy to scratch, keep the accumulation)
            mscr = mid.tile([128, CW], BF16, tag="mscr", name=f"mscr{s}")
            nc.scalar.activation(
                out=mscr, in_=m_t, func=ACTF.Copy,
                accum_out=msum[:, s:s + 1])
            oscr = mid.tile([128, CW], BF16, tag="oscr", name=f"oscr{s}")
            nc.scalar.activation(
                out=oscr, in_=o_t[:, C0:C0 + CW], func=ACTF.Copy,
                accum_out=osum[:, s:s + 1])

            # grad = S_Z(u2) banded matmul; sigmoid edge + accum
            for j in range(3):
                g_t = gps.tile([128, SPAN], F32, tag="g", name=f"g{s}_{j}")
                for k in range(3):
                    off = SPAN * j + 512 * k
                    if fold:
                        nc.tensor.matmul(
                            out=g_t[:, 512 * k:512 * (k + 1)],
                            lhsT=bz_t, rhs=u1_t[:, off:off + 512],
                            start=True, stop=False)
                        nc.tensor.matmul(
                            out=g_t[:, 512 * k:512 * (k + 1)],
                            lhsT=bz_t, rhs=u1_t[:, off + W:off + W + 512],
                            start=False, stop=True)
                    else:
                        nc.tensor.matmul(
                            out=g_t[:, 512 * k:512 * (k + 1)],
                            lhsT=bz_t, rhs=u2_t[:, off:off + 512],
                            start=True, stop=True)
                e_t = mid.tile([128, SPAN], BF16, tag="edge", name=f"e{s}_{j}")
                nc.scalar.activation(
                    out=e_t, in_=g_t, func=ACTF.Sigmoid,
                    scale=100.0, bias=nbias_t,
                    accum_out=edgesum[:, 3 * s + j:3 * s + j + 1])

            # eq / q sums: routed ones-matmul into persistent [12,512] psum
            ones_s = ones_t[:, 12 * s:12 * (s + 1)]
            for k in range(9):
                nc.tensor.matmul(
                    out=eqp, lhsT=ones_s,
                    rhs=eq_t[:, 512 * k:512 * (k + 1)],
                    start=(s == 0 and k == 0), stop=(s == NS - 1 and k == 8))
            for k in range(9):
                nc.tensor.matmul(
                    out=qp, lhsT=ones_s,
                    rhs=q_t[:, 512 * k:512 * (k + 1)],
                    start=(s == 0 and k == 0), stop=(s == NS - 1 and k == 8))

        eqsb = slots.tile([12, 512], F32, name="eqsb")
        nc.vector.tensor_copy(eqsb, eqp)
        nc.default_dma_engine.dma_start(out=eqs_d, in_=eqsb)
        qsb = slots.tile([12, 512], F32, name="qsb")
        nc.vector.tensor_copy(qsb, qp)
        nc.default_dma_engine.dma_start(out=qs_d, in_=qsb)
        nc.default_dma_engine.dma_start(out=part_d[:, 0:6], in_=msum)
        nc.default_dma_engine.dma_start(out=part_d[:, 6:12], in_=osum)
        nc.default_dma_engine.dma_start(out=part_d[:, 12:30], in_=edgesum)

    nc.compile()
    return nc


def _get_program():
    if "nc" not in _CACHE:
        _CACHE["nc"] = _build_program()
    return _CACHE["nc"]


def _make_in_maps(output, masks, loss_threshold):
    o5 = np.asarray(output, dtype=np.float32).reshape(NV, Z, H, W)
    o5b = o5.astype(ml_dtypes.bfloat16)
    m5 = np.asarray(masks, dtype=np.float32).reshape(NV, Z, H, W)
    m5b = m5.astype(ml_dtypes.bfloat16)          # 0/1 exact
    thr = np.full((1, 1), np.float32(np.asarray(loss_threshold)), np.float32)
    bz, ones12 = _consts()
    in_maps = []
    for c in range(NCORES):
        h0 = HC * c
        idx = np.clip(np.arange(h0 - 1, h0 + HC + 1), 0, H - 1)
        o_sh = np.ascontiguousarray(o5b[:, :, idx, :]).reshape(NV * Z, FW)
        m_sh = np.ascontiguousarray(m5b[:, :, h0:h0 + HC, :]).reshape(
            NV * Z, CW)
        in_maps.append({
            "o": o_sh, "m": m_sh, "thr": thr,
            "bz": bz, "ones12": ones12,
        })
    return in_maps


def _combine(results):
    """Host-side tiny reduction: per-core partials -> loss scalar."""
    sum_m = np.zeros(NV)
    sum_o = np.zeros(NV)
    sum_eq = np.zeros(NV)
    sum_om = np.zeros(NV)
    sum_edge = np.zeros(NV)
    for r in results:
        p = np.asarray(r["partials"], dtype=np.float64)
        eqs = np.asarray(r["eqs"], dtype=np.float64)
        qs = np.asarray(r["qs"], dtype=np.float64)
        # partition p: volume = 2s + p//64, z = p%64
        sum_m += p[:, 0:6].reshape(2, 64, NS).sum(1).T.reshape(-1)
        sum_o += p[:, 6:12].reshape(2, 64, NS).sum(1).T.reshape(-1)
        sum_edge += (p[:, 12:30].reshape(2, 64, NS, 3).sum(axis=(1, 3))
                     .T.reshape(-1))
        # eqs/qs: [12, 512], row 2s+r -> volume 2s + r
        sum_eq += eqs.sum(-1)
        sum_om += qs.sum(-1)

    freq = (sum_m / VOX).reshape(B, C)
    med = np.median(freq, axis=1, keepdims=True)
    w0 = 2.0 * med / (freq.min(axis=1, keepdims=True) + 1e-5)
    cw = (med / (freq + 1e-5)) * sum_eq.reshape(B, C) \
        + w0 * sum_edge.reshape(B, C)
    ps1 = sum_om.reshape(B, C)
    ps2 = (sum_o + sum_m).reshape(B, C)
    nom = (cw * ps1).sum(1)
    denom = (cw * ps2 + 1e-7).sum(1)
    loss = (1.0 - 2.0 * nom / denom).sum() / B
    return np.array([loss], dtype=np.float32)


def run(output, masks, loss_threshold, trace=False, **trace_kwargs):
    nc = _get_program()
    in_maps = _make_in_maps(output, masks, loss_threshold)
    res = run_bass_kernel_spmd(nc, in_maps, list(range(NCORES)),
                               trace=trace, **trace_kwargs)
    return _combine(res.results), res


def kernel(output, masks, loss_threshold):
    loss, _ = run(output, masks, loss_threshold)
    return loss


# revision 19
# speedup vs baseline: 1.1054x; 1.1054x over previous
"""Trainium2 Bass kernel for LogWeightedDICELossMultiClass3D (v3).

Input: output (4,3,64,192,192) f32, masks (same), loss_threshold scalar.
Sharding: H=192 split into 8 slabs of 24 rows (one per core, 1-row halo
clamped on host). Device layout per core: 6 supertiles of 128 partitions
(= 2 volumes x 64 z), free dim = H-rows x 192 W flat.

Host ships o and m as bf16 (m is 0/1 so exact; o rounds at ~4e-3 which
perturbs only the threshold comparison for |o-thr|<2e-3 - well inside the
loss tolerance). Per supertile, reduced to per-(volume-half, z) partials:
  sum(m)            ACT copy+accum
  sum(o)            ACT copy+accum
  sum(o*m)          DVE TT mult -> q, summed by routed PE ones-matmul
  sum(ts==m)        DVE TT is_equal -> eq, summed by routed PE ones-matmul
  sum(sobel_edge)   DVE chain d=D_W(ts), u=S_H(d) (2 adds); PE single-pass
                    banded matmul grad=S_Z(u); ACT sigmoid(100g-50)+accum
ts = (o > thr) via DVE tensor_scalar (4x mode, exact 0/1 bf16).
All grad values are small ints -> bf16/psum arithmetic is exact.
eq/q sums accumulate across supertiles into persistent [12,512] PSUM via a
per-supertile routing stationary matrix (row 2s+r = volume 2s+r).
Host combines the tiny partials into the scalar loss.
"""

import numpy as np
import ml_dtypes

import concourse.bacc as bacc
import concourse.bass as bass
import concourse.tile as tile
from concourse import mybir
import concourse.bass_utils as _bu
from concourse.bass_utils import run_bass_kernel_spmd



F32 = mybir.dt.float32
BF16 = mybir.dt.bfloat16
ALU = mybir.AluOpType
ACTF = mybir.ActivationFunctionType

B, C, Z, H, W = 4, 3, 64, 192, 192
NV = B * C            # 12 volumes
NCORES = 8
HC = H // NCORES      # 24 H-rows per core
NS = NV // 2          # 6 supertiles (2 volumes each)
FH = HC + 2           # 26 rows incl halo
FW = FH * W           # 4992 free elements per partition (o / ts / d)
UW = (HC + 1) * W     # 4800 (u1: 25 rows)
CW = HC * W           # 4608 center free elements
C0 = W                # flat offset of center region (row 1)
SPAN = 1536           # grad chunk (3 per supertile), 3 psum banks each
VOX = Z * H * W

_CACHE = {}


def _band64():
    """[1,2,1] Z-smoothing with scipy 'reflect' (np symmetric) ends."""
    M = np.zeros((Z, Z), dtype=np.float64)
    for i in range(Z):
        M[i, i] = 2.0
        if i > 0:
            M[i, i - 1] += 1.0
        else:
            M[i, i] += 1.0
        if i < Z - 1:
            M[i, i + 1] += 1.0
        else:
            M[i, i] += 1.0
    return M


def _consts():
    Bz = _band64()
    blk = np.zeros((128, 128), dtype=np.float64)
    blk[:64, :64] = Bz
    blk[64:, 64:] = Bz
    bz = blk.astype(ml_dtypes.bfloat16)           # weights 1,2 - exact
    # routing matrix: block s is [128, 12] with col 2s+r = 1 on half r
    ones12 = np.zeros((128, 12 * NS), dtype=ml_dtypes.bfloat16)
    for s in range(NS):
        ones12[:64, 12 * s + 2 * s] = 1.0
        ones12[64:, 12 * s + 2 * s + 1] = 1.0
    return bz, ones12


def _build_program():
    nc = bacc.Bacc("TRN2", target_bir_lowering=False, debug=False,
                   num_devices=NCORES)
    o_d = nc.dram_tensor("o", [NV * Z, FW], BF16, kind="ExternalInput").ap()
    m_d = nc.dram_tensor("m", [NV * Z, CW], BF16, kind="ExternalInput").ap()
    thr_d = nc.dram_tensor("thr", [1, 1], F32, kind="ExternalInput").ap()
    bz_d = nc.dram_tensor("bz", [128, 128], BF16, kind="ExternalInput").ap()
    ones_d = nc.dram_tensor("ones12", [128, 12 * NS], BF16,
                            kind="ExternalInput").ap()
    part_d = nc.dram_tensor("partials", [128, 30], F32, kind="ExternalOutput").ap()
    eqs_d = nc.dram_tensor("eqs", [12, 512], F32, kind="ExternalOutput").ap()
    qs_d = nc.dram_tensor("qs", [12, 512], F32, kind="ExternalOutput").ap()

    from contextlib import ExitStack
    with tile.TileContext(nc) as tc, ExitStack() as ctx:
        consts = ctx.enter_context(tc.tile_pool(name="consts", bufs=1))
        io = ctx.enter_context(tc.tile_pool(name="io", bufs=2))
        mid = ctx.enter_context(tc.tile_pool(name="mid", bufs=1))
        xfer = ctx.enter_context(tc.tile_pool(name="xfer", bufs=2))
        slots = ctx.enter_context(tc.tile_pool(name="slots", bufs=1))
        gps = ctx.enter_context(tc.tile_pool(name="gps", bufs=2, space="PSUM"))
        ops = ctx.enter_context(tc.tile_pool(name="ops", bufs=1, space="PSUM"))

        # first supertile's o goes out before anything else (head latency)
        o0_t = io.tile([128, FW], BF16, tag="o", name="o0")
        nc.default_dma_engine.dma_start(
            out=o0_t[0:64, :], in_=o_d[0:64, :])
        nc.default_dma_engine.dma_start(
            out=o0_t[64:128, :], in_=o_d[64:128, :])
        m0_t = io.tile([128, CW], BF16, tag="m", name="m0")
        nc.default_dma_engine.dma_start(out=m0_t, in_=m_d[0:128, :])

        thr_t = consts.tile([128, 1], F32)
        nc.gpsimd.dma_start(out=thr_t, in_=thr_d.to_broadcast([128, 1]))
        bz_t = consts.tile([128, 128], BF16)
        nc.default_dma_engine.dma_start(out=bz_t, in_=bz_d)
        ones_t = consts.tile([128, 12 * NS], BF16)
        nc.default_dma_engine.dma_start(out=ones_t, in_=ones_d)
        nbias_t = consts.tile([128, 1], F32)
        nc.vector.memset(nbias_t, -50.0)

        msum = slots.tile([128, NS], F32)
        osum = slots.tile([128, NS], F32)
        edgesum = slots.tile([128, 3 * NS], F32)
        eqp = ops.tile([12, 512], F32, name="eqp")
        qp = ops.tile([12, 512], F32, name="qp")

        for s in range(NS):
            fold = s % 2 == 1     # fold u2's S_H tap into a 2-pass matmul
            if s == 0:
                o_t, m_t = o0_t, m0_t
            else:
                o_t = io.tile([128, FW], BF16, tag="o", name=f"o{s}")
                nc.default_dma_engine.dma_start(
                    out=o_t[0:64, :], in_=o_d[128 * s:128 * s + 64, :])
                nc.default_dma_engine.dma_start(
                    out=o_t[64:128, :], in_=o_d[128 * s + 64:128 * (s + 1), :])
                m_t = io.tile([128, CW], BF16, tag="m", name=f"m{s}")
                nc.default_dma_engine.dma_start(
                    out=m_t, in_=m_d[128 * s:128 * (s + 1), :])

            ts_t = mid.tile([128, FW], BF16, tag="ts", name=f"ts{s}")
            d_t = mid.tile([128, FW], BF16, tag="d", name=f"d{s}")
            u1_t = mid.tile([128, UW], BF16, tag="u1", name=f"u1{s}")
            if not fold:
                u2_t = xfer.tile([128, CW], BF16, tag="u2", name=f"u2{s}")
            eq_t = xfer.tile([128, CW], BF16, tag="eq", name=f"eq{s}")
            q_t = xfer.tile([128, CW], BF16, tag="q", name=f"q{s}")
            last = s == NS - 1

            # supertile 0 runs per partition-half so DVE starts on the first
            # half of the very first o transfer
            halves = [(0, 64), (64, 128)] if s == 0 else [(0, 128)]
            for (p0, p1) in halves:
                # ts = (o > thr), bf16 0/1 (exact), incl halo rows  [DVE 4x]
                nc.vector.tensor_scalar(
                    out=ts_t[p0:p1], in0=o_t[p0:p1], scalar1=thr_t[p0:p1],
                    scalar2=None, op0=ALU.is_gt)

                if last:
                    # feed the trailing eq/q matmuls as early as possible
                    nc.vector.tensor_tensor(
                        out=eq_t[p0:p1], in0=ts_t[p0:p1, C0:C0 + CW],
                        in1=m_t[p0:p1], op=ALU.is_equal)
                    nc.vector.tensor_tensor(
                        out=q_t[p0:p1], in0=o_t[p0:p1, C0:C0 + CW],
                        in1=m_t[p0:p1], op=ALU.mult)

                # d = W-derivative of ts (symmetric boundary), natural layout
                ts3 = ts_t[p0:p1].rearrange("p (a b) -> p a b", b=W)
                d3 = d_t[p0:p1].rearrange("p (a b) -> p a b", b=W)
                nc.vector.tensor_tensor(
                    out=d3[:, :, 1:191], in0=ts3[:, :, 2:192],
                    in1=ts3[:, :, 0:190], op=ALU.subtract)
                nc.vector.tensor_tensor(
                    out=d3[:, :, 0:192:191], in0=ts3[:, :, 1:192:190],
                    in1=ts3[:, :, 0:191:190], op=ALU.subtract)

                # u1 = d[h]+d[h+1]; u2 = u1[h]+u1[h+1] (= S_H(d)) either on
                # DVE or folded into the S_Z matmul as two accumulating passes
                nc.vector.tensor_tensor(
                    out=u1_t[p0:p1], in0=d_t[p0:p1, 0:UW],
                    in1=d_t[p0:p1, W:UW + W], op=ALU.add)
                if not fold:
                    nc.vector.tensor_tensor(
                        out=u2_t[p0:p1], in0=u1_t[p0:p1, 0:CW],
                        in1=u1_t[p0:p1, W:CW + W], op=ALU.add)

                if not last:
                    nc.vector.tensor_tensor(
                        out=eq_t[p0:p1], in0=ts_t[p0:p1, C0:C0 + CW],
                        in1=m_t[p0:p1], op=ALU.is_equal)
                    nc.vector.tensor_tensor(
                        out=q_t[p0:p1], in0=o_t[p0:p1, C0:C0 + CW],
                        in1=m_t[p0:p1], op=ALU.mult)

            # sum(m), sum(o) on ACT (copy to scratch, keep the accumulation)
            mscr = mid.tile([128, CW], BF16, tag="mscr", name=f"mscr{s}")
            nc.scalar.activation(
                out=mscr, in_=m_t, func=ACTF.Copy,
                accum_out=msum[:, s:s + 1])
            oscr = mid.tile([128, CW], BF16, tag="oscr", name=f"oscr{s}")
            nc.scalar.activation(
                out=oscr, in_=o_t[:, C0:C0 + CW], func=ACTF.Copy,
                accum_out=osum[:, s:s + 1])

            # grad = S_Z(u2) banded matmul; sigmoid edge + accum
            for j in range(3):
                g_t = gps.tile([128, SPAN], F32, tag="g", name=f"g{s}_{j}")
                for k in range(3):
                    off = SPAN * j + 512 * k
                    if fold:
                        nc.tensor.matmul(
                            out=g_t[:, 512 * k:512 * (k + 1)],
                            lhsT=bz_t, rhs=u1_t[:, off:off + 512],
                            start=True, stop=False)
                        nc.tensor.matmul(
                            out=g_t[:, 512 * k:512 * (k + 1)],
                            lhsT=bz_t, rhs=u1_t[:, off + W:off + W + 512],
                            start=False, stop=True)
                    else:
                        nc.tensor.matmul(
                            out=g_t[:, 512 * k:512 * (k + 1)],
                            lhsT=bz_t, rhs=u2_t[:, off:off + 512],
                            start=True, stop=True)
                e_t = mid.tile([128, SPAN], BF16, tag="edge", name=f"e{s}_{j}")
                nc.scalar.activation(
                    out=e_t, in_=g_t, func=ACTF.Sigmoid,
                    scale=100.0, bias=nbias_t,
                    accum_out=edgesum[:, 3 * s + j:3 * s + j + 1])

            # eq / q sums: routed ones-matmul into persistent [12,512] psum
            ones_s = ones_t[:, 12 * s:12 * (s + 1)]
            for k in range(9):
                nc.tensor.matmul(
                    out=eqp, lhsT=ones_s,
                    rhs=eq_t[:, 512 * k:512 * (k + 1)],
                    start=(s == 0 and k == 0), stop=(s == NS - 1 and k == 8))
            for k in range(9):
                nc.tensor.matmul(
                    out=qp, lhsT=ones_s,
                    rhs=q_t[:, 512 * k:512 * (k + 1)],
                    start=(s == 0 and k == 0), stop=(s == NS - 1 and k == 8))

        eqsb = slots.tile([12, 512], F32, name="eqsb")
        nc.vector.tensor_copy(eqsb, eqp)
        nc.default_dma_engine.dma_start(out=eqs_d, in_=eqsb)
        qsb = slots.tile([12, 512], F32, name="qsb")
        nc.vector.tensor_copy(qsb, qp)
        nc.default_dma_engine.dma_start(out=qs_d, in_=qsb)
        nc.default_dma_engine.dma_start(out=part_d[:, 0:6], in_=msum)
        nc.default_dma_engine.dma_start(out=part_d[:, 6:12], in_=osum)
        nc.default_dma_engine.dma_start(out=part_d[:, 12:30], in_=edgesum)

    nc.compile()
    return nc


def _get_program():
    if "nc" not in _CACHE:
        _CACHE["nc"] = _build_program()
    return _CACHE["nc"]


def _make_in_maps(output, masks, loss_threshold):
    o5 = np.asarray(output, dtype=np.float32).reshape(NV, Z, H, W)
    o5b = o5.astype(ml_dtypes.bfloat16)
    m5 = np.asarray(masks, dtype=np.float32).reshape(NV, Z, H, W)
    m5b = m5.astype(ml_dtypes.bfloat16)          # 0/1 exact
    thr = np.full((1, 1), np.float32(np.asarray(loss_threshold)), np.float32)
    bz, ones12 = _consts()
    in_maps = []
    for c in range(NCORES):
        h0 = HC * c
        idx = np.clip(np.arange(h0 - 1, h0 + HC + 1), 0, H - 1)
        o_sh = np.ascontiguousarray(o5b[:, :, idx, :]).reshape(NV * Z, FW)
        m_sh = np.ascontiguousarray(m5b[:, :, h0:h0 + HC, :]).reshape(
            NV * Z, CW)
        in_maps.append({
            "o": o_sh, "m": m_sh, "thr": thr,
            "bz": bz, "ones12": ones12,
        })
    return in_maps


def _combine(results):
    """Host-side tiny reduction: per-core partials -> loss scalar."""
    sum_m = np.zeros(NV)
    sum_o = np.zeros(NV)
    sum_eq = np.zeros(NV)
    sum_om = np.zeros(NV)
    sum_edge = np.zeros(NV)
    for r in results:
        p = np.asarray(r["partials"], dtype=np.float64)
        eqs = np.asarray(r["eqs"], dtype=np.float64)
        qs = np.asarray(r["qs"], dtype=np.float64)
        # partition p: volume = 2s + p//64, z = p%64
        sum_m += p[:, 0:6].reshape(2, 64, NS).sum(1).T.reshape(-1)
        sum_o += p[:, 6:12].reshape(2, 64, NS).sum(1).T.reshape(-1)
        sum_edge += (p[:, 12:30].reshape(2, 64, NS, 3).sum(axis=(1, 3))
                     .T.reshape(-1))
        # eqs/qs: [12, 512], row 2s+r -> volume 2s + r
        sum_eq += eqs.sum(-1)
        sum_om += qs.sum(-1)

    freq = (sum_m / VOX).reshape(B, C)
    med = np.median(freq, axis=1, keepdims=True)
    w0 = 2.0 * med / (freq.min(axis=1, keepdims=True) + 1e-5)
    cw = (med / (freq + 1e-5)) * sum_eq.reshape(B, C) \
        + w0 * sum_edge.reshape(B, C)
    ps1 = sum_om.reshape(B, C)
    ps2 = (sum_o + sum_m).reshape(B, C)
    nom = (cw * ps1).sum(1)
    denom = (cw * ps2 + 1e-7).sum(1)
    loss = (1.0 - 2.0 * nom / denom).sum() / B
    return np.array([loss], dtype=np.float32)


def run(output, masks, loss_threshold, trace=False, **trace_kwargs):
    nc = _get_program()
    in_maps = _make_in_maps(output, masks, loss_threshold)
    res = run_bass_kernel_spmd(nc, in_maps, list(range(NCORES)),
                               trace=trace, **trace_kwargs)
    return _combine(res.results), res


def kernel(output, masks, loss_threshold):
    loss, _ = run(output, masks, loss_threshold)
    return loss


# revision 20
# speedup vs baseline: 1.1947x; 1.0807x over previous
"""Trainium2 Bass kernel for LogWeightedDICELossMultiClass3D (v3).

Input: output (4,3,64,192,192) f32, masks (same), loss_threshold scalar.
Sharding: H=192 split into 8 slabs of 24 rows (one per core, 1-row halo
clamped on host). Device layout per core: 6 supertiles of 128 partitions
(= 2 volumes x 64 z), free dim = H-rows x 192 W flat.

Host ships o and m as bf16 (m is 0/1 so exact; o rounds at ~4e-3 which
perturbs only the threshold comparison for |o-thr|<2e-3 - well inside the
loss tolerance). Per supertile, reduced to per-(volume-half, z) partials:
  sum(m)            ACT copy+accum
  sum(o)            ACT copy+accum
  sum(o*m)          DVE TT mult -> q, summed by routed PE ones-matmul
  sum(ts==m)        DVE TT is_equal -> eq, summed by routed PE ones-matmul
  sum(sobel_edge)   DVE chain d=D_W(ts), u=S_H(d) (2 adds); PE single-pass
                    banded matmul grad=S_Z(u); ACT sigmoid(100g-50)+accum
ts = (o > thr) via DVE tensor_scalar (4x mode, exact 0/1 bf16).
All grad values are small ints -> bf16/psum arithmetic is exact.
eq/q sums accumulate across supertiles into persistent [12,512] PSUM via a
per-supertile routing stationary matrix (row 2s+r = volume 2s+r).
Host combines the tiny partials into the scalar loss.
"""

import numpy as np
import ml_dtypes

import concourse.bacc as bacc
import concourse.bass as bass
import concourse.tile as tile
from concourse import mybir
import concourse.bass_utils as _bu
from concourse.bass_utils import run_bass_kernel_spmd



F32 = mybir.dt.float32
BF16 = mybir.dt.bfloat16
ALU = mybir.AluOpType
ACTF = mybir.ActivationFunctionType

B, C, Z, H, W = 4, 3, 64, 192, 192
NV = B * C            # 12 volumes
NCORES = 8
HC = H // NCORES      # 24 H-rows per core
NS = NV // 2          # 6 supertiles (2 volumes each)
FH = HC + 2           # 26 rows incl halo
FW = FH * W           # 4992 free elements per partition (o / ts / d)
UW = (HC + 1) * W     # 4800 (u1: 25 rows)
CW = HC * W           # 4608 center free elements
C0 = W                # flat offset of center region (row 1)
SPAN = 1536           # grad chunk (3 per supertile), 3 psum banks each
VOX = Z * H * W

_CACHE = {}


def _band64():
    """[1,2,1] Z-smoothing with scipy 'reflect' (np symmetric) ends."""
    M = np.zeros((Z, Z), dtype=np.float64)
    for i in range(Z):
        M[i, i] = 2.0
        if i > 0:
            M[i, i - 1] += 1.0
        else:
            M[i, i] += 1.0
        if i < Z - 1:
            M[i, i + 1] += 1.0
        else:
            M[i, i] += 1.0
    return M


def _consts():
    Bz = _band64()
    blk = np.zeros((128, 128), dtype=np.float64)
    blk[:64, :64] = Bz
    blk[64:, 64:] = Bz
    bz = blk.astype(ml_dtypes.bfloat16)           # weights 1,2 - exact
    # routing matrix: block s is [128, 12] with col 2s+r = 1 on half r
    ones12 = np.zeros((128, 12 * NS), dtype=ml_dtypes.bfloat16)
    for s in range(NS):
        ones12[:64, 12 * s + 2 * s] = 1.0
        ones12[64:, 12 * s + 2 * s + 1] = 1.0
    return bz, ones12


def _build_program():
    nc = bacc.Bacc("TRN2", target_bir_lowering=False, debug=False,
                   num_devices=NCORES)
    o_d = nc.dram_tensor("o", [NV * Z, FW], BF16, kind="ExternalInput").ap()
    m_d = nc.dram_tensor("m", [NV * Z, CW], BF16, kind="ExternalInput").ap()
    thr_d = nc.dram_tensor("thr", [1, 1], F32, kind="ExternalInput").ap()
    bz_d = nc.dram_tensor("bz", [128, 128], BF16, kind="ExternalInput").ap()
    ones_d = nc.dram_tensor("ones12", [128, 12 * NS], BF16,
                            kind="ExternalInput").ap()
    part_d = nc.dram_tensor("partials", [128, 30], F32, kind="ExternalOutput").ap()
    eqs_d = nc.dram_tensor("eqs", [12, 512], F32, kind="ExternalOutput").ap()
    qs_d = nc.dram_tensor("qs", [12, 512], F32, kind="ExternalOutput").ap()

    from contextlib import ExitStack
    with tile.TileContext(nc) as tc, ExitStack() as ctx:
        consts = ctx.enter_context(tc.tile_pool(name="consts", bufs=1))
        io = ctx.enter_context(tc.tile_pool(name="io", bufs=2))
        mid = ctx.enter_context(tc.tile_pool(name="mid", bufs=1))
        xfer = ctx.enter_context(tc.tile_pool(name="xfer", bufs=2))
        slots = ctx.enter_context(tc.tile_pool(name="slots", bufs=1))
        gps = ctx.enter_context(tc.tile_pool(name="gps", bufs=2, space="PSUM"))
        ops = ctx.enter_context(tc.tile_pool(name="ops", bufs=1, space="PSUM"))

        # first supertile's o goes out before anything else (head latency)
        o0_t = io.tile([128, FW], BF16, tag="o", name="o0")
        nc.default_dma_engine.dma_start(
            out=o0_t[0:64, :], in_=o_d[0:64, :])
        nc.default_dma_engine.dma_start(
            out=o0_t[64:128, :], in_=o_d[64:128, :])
        m0_t = io.tile([128, CW], BF16, tag="m", name="m0")
        nc.default_dma_engine.dma_start(out=m0_t, in_=m_d[0:128, :])

        thr_t = consts.tile([128, 1], F32)
        nc.gpsimd.dma_start(out=thr_t, in_=thr_d.to_broadcast([128, 1]))
        bz_t = consts.tile([128, 128], BF16)
        nc.default_dma_engine.dma_start(out=bz_t, in_=bz_d)
        ones_t = consts.tile([128, 12 * NS], BF16)
        nc.default_dma_engine.dma_start(out=ones_t, in_=ones_d)
        nbias_t = consts.tile([128, 1], F32)
        nc.vector.memset(nbias_t, -50.0)

        msum = slots.tile([128, NS], F32)
        osum = slots.tile([128, NS], F32)
        edgesum = slots.tile([128, 3 * NS], F32)
        eqp = ops.tile([12, 512], F32, name="eqp")
        qp = ops.tile([12, 512], F32, name="qp")

        for s in range(NS):
            fold = s % 2 == 1     # fold u2's S_H tap into a 2-pass matmul
            if s == 0:
                o_t, m_t = o0_t, m0_t
            else:
                o_t = io.tile([128, FW], BF16, tag="o", name=f"o{s}")
                nc.default_dma_engine.dma_start(
                    out=o_t[0:64, :], in_=o_d[128 * s:128 * s + 64, :])
                nc.default_dma_engine.dma_start(
                    out=o_t[64:128, :], in_=o_d[128 * s + 64:128 * (s + 1), :])
                m_t = io.tile([128, CW], BF16, tag="m", name=f"m{s}")
                nc.default_dma_engine.dma_start(
                    out=m_t, in_=m_d[128 * s:128 * (s + 1), :])

            ts_t = mid.tile([128, FW], BF16, tag="ts", name=f"ts{s}")
            d_t = mid.tile([128, FW], BF16, tag="d", name=f"d{s}")
            u1_t = mid.tile([128, UW], BF16, tag="u1", name=f"u1{s}")
            if not fold:
                u2_t = xfer.tile([128, CW], BF16, tag="u2", name=f"u2{s}")
            eq_t = xfer.tile([128, CW], BF16, tag="eq", name=f"eq{s}")
            q_t = xfer.tile([128, CW], BF16, tag="q", name=f"q{s}")
            last = s == NS - 1

            halves = [(0, 128)]
            for (p0, p1) in halves:
                # ts = (o > thr), bf16 0/1 (exact), incl halo rows  [DVE 4x]
                nc.vector.tensor_scalar(
                    out=ts_t[p0:p1], in0=o_t[p0:p1], scalar1=thr_t[p0:p1],
                    scalar2=None, op0=ALU.is_gt)

                if last:
                    # feed the trailing eq/q matmuls as early as possible
                    nc.vector.tensor_tensor(
                        out=eq_t[p0:p1], in0=ts_t[p0:p1, C0:C0 + CW],
                        in1=m_t[p0:p1], op=ALU.is_equal)
                    nc.vector.tensor_tensor(
                        out=q_t[p0:p1], in0=o_t[p0:p1, C0:C0 + CW],
                        in1=m_t[p0:p1], op=ALU.mult)

                # d = W-derivative of ts (symmetric boundary), natural layout
                ts3 = ts_t[p0:p1].rearrange("p (a b) -> p a b", b=W)
                d3 = d_t[p0:p1].rearrange("p (a b) -> p a b", b=W)
                nc.vector.tensor_tensor(
                    out=d3[:, :, 1:191], in0=ts3[:, :, 2:192],
                    in1=ts3[:, :, 0:190], op=ALU.subtract)
                nc.vector.tensor_tensor(
                    out=d3[:, :, 0:192:191], in0=ts3[:, :, 1:192:190],
                    in1=ts3[:, :, 0:191:190], op=ALU.subtract)

                # u1 = d[h]+d[h+1]; u2 = u1[h]+u1[h+1] (= S_H(d)) either on
                # DVE or folded into the S_Z matmul as two accumulating passes
                nc.vector.tensor_tensor(
                    out=u1_t[p0:p1], in0=d_t[p0:p1, 0:UW],
                    in1=d_t[p0:p1, W:UW + W], op=ALU.add)
                if not fold:
                    nc.vector.tensor_tensor(
                        out=u2_t[p0:p1], in0=u1_t[p0:p1, 0:CW],
                        in1=u1_t[p0:p1, W:CW + W], op=ALU.add)

                if not last:
                    nc.vector.tensor_tensor(
                        out=eq_t[p0:p1], in0=ts_t[p0:p1, C0:C0 + CW],
                        in1=m_t[p0:p1], op=ALU.is_equal)
                    nc.vector.tensor_tensor(
                        out=q_t[p0:p1], in0=o_t[p0:p1, C0:C0 + CW],
                        in1=m_t[p0:p1], op=ALU.mult)

            # sum(m), sum(o) on ACT (copy to scratch, keep the accumulation)
            mscr = mid.tile([128, CW], BF16, tag="mscr", name=f"mscr{s}")
            nc.scalar.activation(
                out=mscr, in_=m_t, func=ACTF.Copy,
                accum_out=msum[:, s:s + 1])
            oscr = mid.tile([128, CW], BF16, tag="oscr", name=f"oscr{s}")
            nc.scalar.activation(
                out=oscr, in_=o_t[:, C0:C0 + CW], func=ACTF.Copy,
                accum_out=osum[:, s:s + 1])

            # grad = S_Z(u2) banded matmul; sigmoid edge + accum
            for j in range(3):
                g_t = gps.tile([128, SPAN], F32, tag="g", name=f"g{s}_{j}")
                for k in range(3):
                    off = SPAN * j + 512 * k
                    if fold:
                        nc.tensor.matmul(
                            out=g_t[:, 512 * k:512 * (k + 1)],
                            lhsT=bz_t, rhs=u1_t[:, off:off + 512],
                            start=True, stop=False)
                        nc.tensor.matmul(
                            out=g_t[:, 512 * k:512 * (k + 1)],
                            lhsT=bz_t, rhs=u1_t[:, off + W:off + W + 512],
                            start=False, stop=True)
                    else:
                        nc.tensor.matmul(
                            out=g_t[:, 512 * k:512 * (k + 1)],
                            lhsT=bz_t, rhs=u2_t[:, off:off + 512],
                            start=True, stop=True)
                e_t = mid.tile([128, SPAN], BF16, tag="edge", name=f"e{s}_{j}")
                nc.scalar.activation(
                    out=e_t, in_=g_t, func=ACTF.Sigmoid,
                    scale=100.0, bias=nbias_t,
                    accum_out=edgesum[:, 3 * s + j:3 * s + j + 1])

            # eq / q sums: routed ones-matmul into persistent [12,512] psum
            ones_s = ones_t[:, 12 * s:12 * (s + 1)]
            for k in range(9):
                nc.tensor.matmul(
                    out=eqp, lhsT=ones_s,
                    rhs=eq_t[:, 512 * k:512 * (k + 1)],
                    start=(s == 0 and k == 0), stop=(s == NS - 1 and k == 8))
            for k in range(9):
                nc.tensor.matmul(
                    out=qp, lhsT=ones_s,
                    rhs=q_t[:, 512 * k:512 * (k + 1)],
                    start=(s == 0 and k == 0), stop=(s == NS - 1 and k == 8))

        eqsb = slots.tile([12, 512], F32, name="eqsb")
        nc.vector.tensor_copy(eqsb, eqp)
        nc.default_dma_engine.dma_start(out=eqs_d, in_=eqsb)
        qsb = slots.tile([12, 512], F32, name="qsb")
        nc.vector.tensor_copy(qsb, qp)
        nc.default_dma_engine.dma_start(out=qs_d, in_=qsb)
        nc.default_dma_engine.dma_start(out=part_d[:, 0:6], in_=msum)
        nc.default_dma_engine.dma_start(out=part_d[:, 6:12], in_=osum)
        nc.default_dma_engine.dma_start(out=part_d[:, 12:30], in_=edgesum)

    nc.compile()
    return nc


def _get_program():
    if "nc" not in _CACHE:
        _CACHE["nc"] = _build_program()
    return _CACHE["nc"]


def _make_in_maps(output, masks, loss_threshold):
    o5 = np.asarray(output, dtype=np.float32).reshape(NV, Z, H, W)
    o5b = o5.astype(ml_dtypes.bfloat16)
    m5 = np.asarray(masks, dtype=np.float32).reshape(NV, Z, H, W)
    m5b = m5.astype(ml_dtypes.bfloat16)          # 0/1 exact
    thr = np.full((1, 1), np.float32(np.asarray(loss_threshold)), np.float32)
    bz, ones12 = _consts()
    in_maps = []
    for c in range(NCORES):
        h0 = HC * c
        idx = np.clip(np.arange(h0 - 1, h0 + HC + 1), 0, H - 1)
        o_sh = np.ascontiguousarray(o5b[:, :, idx, :]).reshape(NV * Z, FW)
        m_sh = np.ascontiguousarray(m5b[:, :, h0:h0 + HC, :]).reshape(
            NV * Z, CW)
        in_maps.append({
            "o": o_sh, "m": m_sh, "thr": thr,
            "bz": bz, "ones12": ones12,
        })
    return in_maps


def _combine(results):
    """Host-side tiny reduction: per-core partials -> loss scalar."""
    sum_m = np.zeros(NV)
    sum_o = np.zeros(NV)
    sum_eq = np.zeros(NV)
    sum_om = np.zeros(NV)
    sum_edge = np.zeros(NV)
    for r in results:
        p = np.asarray(r["partials"], dtype=np.float64)
        eqs = np.asarray(r["eqs"], dtype=np.float64)
        qs = np.asarray(r["qs"], dtype=np.float64)
        # partition p: volume = 2s + p//64, z = p%64
        sum_m += p[:, 0:6].reshape(2, 64, NS).sum(1).T.reshape(-1)
        sum_o += p[:, 6:12].reshape(2, 64, NS).sum(1).T.reshape(-1)
        sum_edge += (p[:, 12:30].reshape(2, 64, NS, 3).sum(axis=(1, 3))
                     .T.reshape(-1))
        # eqs/qs: [12, 512], row 2s+r -> volume 2s + r
        sum_eq += eqs.sum(-1)
        sum_om += qs.sum(-1)

    freq = (sum_m / VOX).reshape(B, C)
    med = np.median(freq, axis=1, keepdims=True)
    w0 = 2.0 * med / (freq.min(axis=1, keepdims=True) + 1e-5)
    cw = (med / (freq + 1e-5)) * sum_eq.reshape(B, C) \
        + w0 * sum_edge.reshape(B, C)
    ps1 = sum_om.reshape(B, C)
    ps2 = (sum_o + sum_m).reshape(B, C)
    nom = (cw * ps1).sum(1)
    denom = (cw * ps2 + 1e-7).sum(1)
    loss = (1.0 - 2.0 * nom / denom).sum() / B
    return np.array([loss], dtype=np.float32)


def run(output, masks, loss_threshold, trace=False, **trace_kwargs):
    nc = _get_program()
    in_maps = _make_in_maps(output, masks, loss_threshold)
    res = run_bass_kernel_spmd(nc, in_maps, list(range(NCORES)),
                               trace=trace, **trace_kwargs)
    return _combine(res.results), res


def kernel(output, masks, loss_threshold):
    loss, _ = run(output, masks, loss_threshold)
    return loss


# revision 23
# speedup vs baseline: 1.2177x; 1.0192x over previous
"""Trainium2 Bass kernel for LogWeightedDICELossMultiClass3D (v3).

Input: output (4,3,64,192,192) f32, masks (same), loss_threshold scalar.
Sharding: H=192 split into 8 slabs of 24 rows (one per core, 1-row halo
clamped on host). Device layout per core: 6 supertiles of 128 partitions
(= 2 volumes x 64 z), free dim = H-rows x 192 W flat.

Host ships o and m as bf16 (m is 0/1 so exact; o rounds at ~4e-3 which
perturbs only the threshold comparison for |o-thr|<2e-3 - well inside the
loss tolerance). Per supertile, reduced to per-(volume-half, z) partials:
  sum(m)            ACT copy+accum
  sum(o)            ACT copy+accum
  sum(o*m)          DVE TT mult -> q, summed by routed PE ones-matmul
  sum(ts==m)        DVE TT is_equal -> eq, summed by routed PE ones-matmul
  sum(sobel_edge)   DVE chain d=D_W(ts), u=S_H(d) (2 adds); PE single-pass
                    banded matmul grad=S_Z(u); ACT sigmoid(100g-50)+accum
ts = (o > thr) via DVE tensor_scalar (4x mode, exact 0/1 bf16).
All grad values are small ints -> bf16/psum arithmetic is exact.
eq/q sums accumulate across supertiles into persistent [12,512] PSUM via a
per-supertile routing stationary matrix (row 2s+r = volume 2s+r).
Host combines the tiny partials into the scalar loss.
"""

import numpy as np
import ml_dtypes

import concourse.bacc as bacc
import concourse.bass as bass
import concourse.tile as tile
from concourse import mybir
import concourse.bass_utils as _bu
from concourse.bass_utils import run_bass_kernel_spmd



F32 = mybir.dt.float32
BF16 = mybir.dt.bfloat16
ALU = mybir.AluOpType
ACTF = mybir.ActivationFunctionType

B, C, Z, H, W = 4, 3, 64, 192, 192
NV = B * C            # 12 volumes
NCORES = 8
HC = H // NCORES      # 24 H-rows per core
NS = NV // 2          # 6 supertiles (2 volumes each)
FH = HC + 2           # 26 rows incl halo
FW = FH * W           # 4992 free elements per partition (o / ts / d)
UW = (HC + 1) * W     # 4800 (u1: 25 rows)
CW = HC * W           # 4608 center free elements
C0 = W                # flat offset of center region (row 1)
SPAN = 1536           # grad chunk (3 per supertile), 3 psum banks each
VOX = Z * H * W

_CACHE = {}


def _band64():
    """[1,2,1] Z-smoothing with scipy 'reflect' (np symmetric) ends."""
    M = np.zeros((Z, Z), dtype=np.float64)
    for i in range(Z):
        M[i, i] = 2.0
        if i > 0:
            M[i, i - 1] += 1.0
        else:
            M[i, i] += 1.0
        if i < Z - 1:
            M[i, i + 1] += 1.0
        else:
            M[i, i] += 1.0
    return M


def _consts():
    Bz = _band64()
    blk = np.zeros((128, 128), dtype=np.float64)
    blk[:64, :64] = Bz
    blk[64:, 64:] = Bz
    bz = blk.astype(ml_dtypes.bfloat16)           # weights 1,2 - exact
    # routing matrix: block s is [128, 12] with col 2s+r = 1 on half r
    ones12 = np.zeros((128, 12 * NS), dtype=ml_dtypes.bfloat16)
    for s in range(NS):
        ones12[:64, 12 * s + 2 * s] = 1.0
        ones12[64:, 12 * s + 2 * s + 1] = 1.0
    return bz, ones12


def _build_program():
    nc = bacc.Bacc("TRN2", target_bir_lowering=False, debug=False,
                   num_devices=NCORES)
    o_d = nc.dram_tensor("o", [NV * Z, FW], BF16, kind="ExternalInput").ap()
    m_d = nc.dram_tensor("m", [NV * Z, CW], BF16, kind="ExternalInput").ap()
    thr_d = nc.dram_tensor("thr", [1, 1], F32, kind="ExternalInput").ap()
    bz_d = nc.dram_tensor("bz", [128, 128], BF16, kind="ExternalInput").ap()
    ones_d = nc.dram_tensor("ones12", [128, 12 * NS], BF16,
                            kind="ExternalInput").ap()
    part_d = nc.dram_tensor("partials", [128, 30], F32, kind="ExternalOutput").ap()
    eqs_d = nc.dram_tensor("eqs", [12, 512], F32, kind="ExternalOutput").ap()
    qs_d = nc.dram_tensor("qs", [12, 512], F32, kind="ExternalOutput").ap()

    from contextlib import ExitStack
    with tile.TileContext(nc) as tc, ExitStack() as ctx:
        consts = ctx.enter_context(tc.tile_pool(name="consts", bufs=1))
        io = ctx.enter_context(tc.tile_pool(name="io", bufs=2))
        mid = ctx.enter_context(tc.tile_pool(name="mid", bufs=1))
        xfer = ctx.enter_context(tc.tile_pool(name="xfer", bufs=2))
        slots = ctx.enter_context(tc.tile_pool(name="slots", bufs=1))
        gps = ctx.enter_context(tc.tile_pool(name="gps", bufs=2, space="PSUM"))
        ops = ctx.enter_context(tc.tile_pool(name="ops", bufs=1, space="PSUM"))

        # first supertile's o goes out before anything else (head latency),
        # split by columns so the first ts half-op can start early
        CH = FW // 2
        o0_t = io.tile([128, FW], BF16, tag="o", name="o0")
        nc.default_dma_engine.dma_start(
            out=o0_t[:, 0:CH], in_=o_d[0:128, 0:CH])
        nc.default_dma_engine.dma_start(
            out=o0_t[:, CH:FW], in_=o_d[0:128, CH:FW])
        m0_t = io.tile([128, CW], BF16, tag="m", name="m0")
        nc.default_dma_engine.dma_start(out=m0_t, in_=m_d[0:128, :])

        thr_t = consts.tile([128, 1], F32)
        nc.gpsimd.dma_start(out=thr_t, in_=thr_d.to_broadcast([128, 1]))
        bz_t = consts.tile([128, 128], BF16)
        nc.default_dma_engine.dma_start(out=bz_t, in_=bz_d)
        ones_t = consts.tile([128, 12 * NS], BF16)
        nc.default_dma_engine.dma_start(out=ones_t, in_=ones_d)
        nbias_t = consts.tile([128, 1], F32)
        nc.vector.memset(nbias_t, -50.0)

        msum = slots.tile([128, NS], F32)
        osum = slots.tile([128, NS], F32)
        edgesum = slots.tile([128, 3 * NS], F32)
        eqp = ops.tile([12, 512], F32, name="eqp")
        qp = ops.tile([12, 512], F32, name="qp")

        for s in range(NS):
            fold = s in (1, 3)    # fold u2's S_H tap into a 2-pass matmul
            if s == 0:
                o_t, m_t = o0_t, m0_t
            else:
                o_t = io.tile([128, FW], BF16, tag="o", name=f"o{s}")
                nc.default_dma_engine.dma_start(
                    out=o_t[0:64, :], in_=o_d[128 * s:128 * s + 64, :])
                nc.default_dma_engine.dma_start(
                    out=o_t[64:128, :], in_=o_d[128 * s + 64:128 * (s + 1), :])
                m_t = io.tile([128, CW], BF16, tag="m", name=f"m{s}")
                nc.default_dma_engine.dma_start(
                    out=m_t, in_=m_d[128 * s:128 * (s + 1), :])

            ts_t = mid.tile([128, FW], BF16, tag="ts", name=f"ts{s}")
            d_t = mid.tile([128, FW], BF16, tag="d", name=f"d{s}")
            u1_t = mid.tile([128, UW], BF16, tag="u1", name=f"u1{s}")
            if not fold:
                u2_t = xfer.tile([128, CW], BF16, tag="u2", name=f"u2{s}")
            eq_t = xfer.tile([128, CW], BF16, tag="eq", name=f"eq{s}")
            q_t = xfer.tile([128, CW], BF16, tag="q", name=f"q{s}")
            last = s == NS - 1

            halves = [(0, 128)]
            for (p0, p1) in halves:
                # ts = (o > thr), bf16 0/1 (exact), incl halo rows  [DVE 4x]
                if s == 0:
                    nc.vector.tensor_scalar(
                        out=ts_t[:, 0:CH], in0=o_t[:, 0:CH], scalar1=thr_t,
                        scalar2=None, op0=ALU.is_gt)
                    nc.vector.tensor_scalar(
                        out=ts_t[:, CH:FW], in0=o_t[:, CH:FW], scalar1=thr_t,
                        scalar2=None, op0=ALU.is_gt)
                else:
                    nc.vector.tensor_scalar(
                        out=ts_t[p0:p1], in0=o_t[p0:p1], scalar1=thr_t[p0:p1],
                        scalar2=None, op0=ALU.is_gt)

                if last:
                    # feed the trailing eq/q matmuls as early as possible
                    nc.vector.tensor_tensor(
                        out=eq_t[p0:p1], in0=ts_t[p0:p1, C0:C0 + CW],
                        in1=m_t[p0:p1], op=ALU.is_equal)
                    nc.vector.tensor_tensor(
                        out=q_t[p0:p1], in0=o_t[p0:p1, C0:C0 + CW],
                        in1=m_t[p0:p1], op=ALU.mult)

                # d = W-derivative of ts (symmetric boundary), natural layout
                ts3 = ts_t[p0:p1].rearrange("p (a b) -> p a b", b=W)
                d3 = d_t[p0:p1].rearrange("p (a b) -> p a b", b=W)
                nc.vector.tensor_tensor(
                    out=d3[:, :, 1:191], in0=ts3[:, :, 2:192],
                    in1=ts3[:, :, 0:190], op=ALU.subtract)
                nc.vector.tensor_tensor(
                    out=d3[:, :, 0:192:191], in0=ts3[:, :, 1:192:190],
                    in1=ts3[:, :, 0:191:190], op=ALU.subtract)

                # u1 = d[h]+d[h+1]; u2 = u1[h]+u1[h+1] (= S_H(d)) either on
                # DVE or folded into the S_Z matmul as two accumulating passes
                nc.vector.tensor_tensor(
                    out=u1_t[p0:p1], in0=d_t[p0:p1, 0:UW],
                    in1=d_t[p0:p1, W:UW + W], op=ALU.add)
                if not fold:
                    nc.vector.tensor_tensor(
                        out=u2_t[p0:p1], in0=u1_t[p0:p1, 0:CW],
                        in1=u1_t[p0:p1, W:CW + W], op=ALU.add)

                if not last:
                    nc.vector.tensor_tensor(
                        out=eq_t[p0:p1], in0=ts_t[p0:p1, C0:C0 + CW],
                        in1=m_t[p0:p1], op=ALU.is_equal)
                    nc.vector.tensor_tensor(
                        out=q_t[p0:p1], in0=o_t[p0:p1, C0:C0 + CW],
                        in1=m_t[p0:p1], op=ALU.mult)

            # sum(m), sum(o) on ACT (copy to scratch, keep the accumulation)
            mscr = mid.tile([128, CW], BF16, tag="mscr", name=f"mscr{s}")
            nc.scalar.activation(
                out=mscr, in_=m_t, func=ACTF.Copy,
                accum_out=msum[:, s:s + 1])
            oscr = mid.tile([128, CW], BF16, tag="oscr", name=f"oscr{s}")
            nc.scalar.activation(
                out=oscr, in_=o_t[:, C0:C0 + CW], func=ACTF.Copy,
                accum_out=osum[:, s:s + 1])

            # grad = S_Z(u2) banded matmul; sigmoid edge + accum
            for j in range(3):
                g_t = gps.tile([128, SPAN], F32, tag="g", name=f"g{s}_{j}")
                for k in range(3):
                    off = SPAN * j + 512 * k
                    if fold:
                        nc.tensor.matmul(
                            out=g_t[:, 512 * k:512 * (k + 1)],
                            lhsT=bz_t, rhs=u1_t[:, off:off + 512],
                            start=True, stop=False)
                        nc.tensor.matmul(
                            out=g_t[:, 512 * k:512 * (k + 1)],
                            lhsT=bz_t, rhs=u1_t[:, off + W:off + W + 512],
                            start=False, stop=True)
                    else:
                        nc.tensor.matmul(
                            out=g_t[:, 512 * k:512 * (k + 1)],
                            lhsT=bz_t, rhs=u2_t[:, off:off + 512],
                            start=True, stop=True)
                e_t = mid.tile([128, SPAN], BF16, tag="edge", name=f"e{s}_{j}")
                nc.scalar.activation(
                    out=e_t, in_=g_t, func=ACTF.Sigmoid,
                    scale=100.0, bias=nbias_t,
                    accum_out=edgesum[:, 3 * s + j:3 * s + j + 1])

            # eq / q sums: routed ones-matmul into persistent [12,512] psum
            ones_s = ones_t[:, 12 * s:12 * (s + 1)]
            for k in range(9):
                nc.tensor.matmul(
                    out=eqp, lhsT=ones_s,
                    rhs=eq_t[:, 512 * k:512 * (k + 1)],
                    start=(s == 0 and k == 0), stop=(s == NS - 1 and k == 8))
            for k in range(9):
                nc.tensor.matmul(
                    out=qp, lhsT=ones_s,
                    rhs=q_t[:, 512 * k:512 * (k + 1)],
                    start=(s == 0 and k == 0), stop=(s == NS - 1 and k == 8))

        eqsb = slots.tile([12, 512], F32, name="eqsb")
        nc.vector.tensor_copy(eqsb, eqp)
        nc.default_dma_engine.dma_start(out=eqs_d, in_=eqsb)
        qsb = slots.tile([12, 512], F32, name="qsb")
        nc.vector.tensor_copy(qsb, qp)
        nc.default_dma_engine.dma_start(out=qs_d, in_=qsb)
        nc.default_dma_engine.dma_start(out=part_d[:, 0:6], in_=msum)
        nc.default_dma_engine.dma_start(out=part_d[:, 6:12], in_=osum)
        nc.default_dma_engine.dma_start(out=part_d[:, 12:30], in_=edgesum)

    nc.compile()
    return nc


def _get_program():
    if "nc" not in _CACHE:
        _CACHE["nc"] = _build_program()
    return _CACHE["nc"]


def _make_in_maps(output, masks, loss_threshold):
    o5 = np.asarray(output, dtype=np.float32).reshape(NV, Z, H, W)
    o5b = o5.astype(ml_dtypes.bfloat16)
    m5 = np.asarray(masks, dtype=np.float32).reshape(NV, Z, H, W)
    m5b = m5.astype(ml_dtypes.bfloat16)          # 0/1 exact
    thr = np.full((1, 1), np.float32(np.asarray(loss_threshold)), np.float32)
    bz, ones12 = _consts()
    in_maps = []
    for c in range(NCORES):
        h0 = HC * c
        idx = np.clip(np.arange(h0 - 1, h0 + HC + 1), 0, H - 1)
        o_sh = np.ascontiguousarray(o5b[:, :, idx, :]).reshape(NV * Z, FW)
        m_sh = np.ascontiguousarray(m5b[:, :, h0:h0 + HC, :]).reshape(
            NV * Z, CW)
        in_maps.append({
            "o": o_sh, "m": m_sh, "thr": thr,
            "bz": bz, "ones12": ones12,
        })
    return in_maps


def _combine(results):
    """Host-side tiny reduction: per-core partials -> loss scalar."""
    sum_m = np.zeros(NV)
    sum_o = np.zeros(NV)
    sum_eq = np.zeros(NV)
    sum_om = np.zeros(NV)
    sum_edge = np.zeros(NV)
    for r in results:
        p = np.asarray(r["partials"], dtype=np.float64)
        eqs = np.asarray(r["eqs"], dtype=np.float64)
        qs = np.asarray(r["qs"], dtype=np.float64)
        # partition p: volume = 2s + p//64, z = p%64
        sum_m += p[:, 0:6].reshape(2, 64, NS).sum(1).T.reshape(-1)
        sum_o += p[:, 6:12].reshape(2, 64, NS).sum(1).T.reshape(-1)
        sum_edge += (p[:, 12:30].reshape(2, 64, NS, 3).sum(axis=(1, 3))
                     .T.reshape(-1))
        # eqs/qs: [12, 512], row 2s+r -> volume 2s + r
        sum_eq += eqs.sum(-1)
        sum_om += qs.sum(-1)

    freq = (sum_m / VOX).reshape(B, C)
    med = np.median(freq, axis=1, keepdims=True)
    w0 = 2.0 * med / (freq.min(axis=1, keepdims=True) + 1e-5)
    cw = (med / (freq + 1e-5)) * sum_eq.reshape(B, C) \
        + w0 * sum_edge.reshape(B, C)
    ps1 = sum_om.reshape(B, C)
    ps2 = (sum_o + sum_m).reshape(B, C)
    nom = (cw * ps1).sum(1)
    denom = (cw * ps2 + 1e-7).sum(1)
    loss = (1.0 - 2.0 * nom / denom).sum() / B
    return np.array([loss], dtype=np.float32)


def run(output, masks, loss_threshold, trace=False, **trace_kwargs):
    nc = _get_program()
    in_maps = _make_in_maps(output, masks, loss_threshold)
    res = run_bass_kernel_spmd(nc, in_maps, list(range(NCORES)),
                               trace=trace, **trace_kwargs)
    return _combine(res.results), res


def kernel(output, masks, loss_threshold):
    loss, _ = run(output, masks, loss_threshold)
    return loss
